# revision 27
# baseline (speedup 1.0000x reference)
"""ButterflyBlock sparse-attention kernel for 8 Trainium2 NeuronCores.

Full inputs in, full output out. The P*B = 32 butterfly blocks are
data-parallel: 4 blocks per core, chunk gather/scatter host-side.

Software-pipelined schedule per core (blocks b = 0..3):
  - QKVO weights are SBUF-resident (loaded once).
  - attn(b) emission weaves in, as "filler" matmuls, the Q/K/V
    projections of block b+1 and the Wo projection of block b-1, so the
    PE never idles on the scores->exp->PV dependency chain.
  - scores for a head pair run as two row-tiled (64-partition) matmuls
    on opposite PE row halves -> concurrent execution.
  - exp on the scalar engine ([128,2,512] per m-chunk), PSUM evacuation
    of y on the scalar engine, q/k/v/u on the vector engine.

Hardcoded shape: x [4, 4096, 1024], D=1024, H=16, dh=64, CHUNK=256,
blocks of L=512; layer_bit picks the chunk pairing (host-side).
"""

import sys

sys.path.insert(0, "/root/.axon_site/_ro/trn_rl_repo")
sys.path.insert(0, "/opt/trn_rl_repo")

import ml_dtypes
import numpy as np

import concourse.bass as bass
import concourse.bacc as bacc
import concourse.mybir as mybir
import concourse.tile as tile
from concourse.bass_utils import run_bass_kernel_spmd

F32 = mybir.dt.float32
BF16 = mybir.dt.bfloat16

B, N, D = 4, 4096, 1024
H, DH = 16, 64
CHUNK = 256
L = 2 * CHUNK          # 512 tokens per block
NBLK = 4               # blocks per core
NCORES = 8
KC = D // 128          # 8 contraction chunks
LC = L // 128          # 4 token chunks
VW = H * 128           # v_sb free width: 16 head-blocks of 128 cols
EXP_FUNC = mybir.ActivationFunctionType.Exp

# filler pump sizes (units of ~2 matmuls each)
PUMP_STEP = 2          # after each exp, before PV
PUMP_PAIR = 3          # at pair boundary (covers u-evac DVE latency)


def _build_nc():
    nc = bacc.Bacc("TRN2", target_bir_lowering=False, debug=False)

    zt_d = nc.dram_tensor("zt", [NBLK, D, L], BF16, kind="ExternalInput")
    wq_d = nc.dram_tensor("wq", [D, D], BF16, kind="ExternalInput")
    wk_d = nc.dram_tensor("wk", [D, D], BF16, kind="ExternalInput")
    wv_d = nc.dram_tensor("wv", [D, D], BF16, kind="ExternalInput")
    wo_d = nc.dram_tensor("wo", [D, D], BF16, kind="ExternalInput")
    ones_d = nc.dram_tensor("ones", [128, 64], BF16, kind="ExternalInput")
    y_d = nc.dram_tensor("y", [NBLK, L, D], BF16, kind="ExternalOutput")

    with tile.TileContext(nc) as tc:
        with (
            tc.tile_pool(name="wsb", bufs=1) as wpool,
            tc.tile_pool(name="zsb", bufs=2) as zpool,
            tc.tile_pool(name="qksb", bufs=2) as qkpool,
            tc.tile_pool(name="vsb", bufs=1) as vpool,
            tc.tile_pool(name="usb", bufs=3) as upool,
            tc.tile_pool(name="psb", bufs=4) as ppool,
            tc.tile_pool(name="ysb", bufs=3) as ypool,
            tc.tile_pool(name="rsb", bufs=2) as rpool,
            tc.tile_pool(name="wrm", bufs=1) as wmpool,
            tc.tile_pool(name="ps", bufs=1, space="PSUM") as pspool,
        ):
            # ---------------- PE warm-up (HAM) during initial DMA wait
            wm = wmpool.tile([128, 640], BF16, tag="wm", name="wm")
            nc.vector.memset(wm[:], 0)
            wm_ps = pspool.tile([128, 512], F32, tag="pj", bufs=2, name="wmps")
            for _ in range(16):
                nc.tensor.matmul(
                    wm_ps[:], wm[:, 0:128].opt(), wm[:, 128:640].opt(),
                    start=True, stop=True,
                )
            # preload the exp activation table (one-time ~1.3us) off the
            # critical path
            wm_exp = wmpool.tile([128, 8], BF16, tag="wme", name="wme")
            nc.scalar.activation(wm_exp[:], wm[:, 0:8], EXP_FUNC)

            # ---------------- resident weights
            w_sb = {}
            for nm, w_dram in (("q", wq_d), ("k", wk_d), ("v", wv_d), ("o", wo_d)):
                t = wpool.tile([128, KC, D], BF16, tag=f"w{nm}", name=f"w{nm}sb")
                w_sb[nm] = t
            # zt block 0 first (SWDGE so it doesn't queue behind weights)
            zt_tiles = {}

            def load_zt(j, eng):
                t = zpool.tile([128, KC, L], BF16, tag="z", name=f"zt{j}")
                zr = zt_d[j].rearrange("(kc p) l -> p kc l", p=128)
                for kc in range(KC):
                    eng.dma_start(t[:, kc, :], zr[:, kc, :])
                zt_tiles[j] = t

            load_zt(0, nc.gpsimd)
            for nm, w_dram in (("q", wq_d), ("k", wk_d), ("v", wv_d), ("o", wo_d)):
                w_r = w_dram.rearrange("(kc p) d -> p kc d", p=128)
                for kc in range(KC):
                    nc.sync.dma_start(w_sb[nm][:, kc, :], w_r[:, kc, :])
            load_zt(1, nc.sync)

            # ---------------- fixed v tiles with ones margins (written once)
            ones_b = bass.AP(
                tensor=ones_d[:].tensor, offset=ones_d[:].offset,
                ap=[list(ones_d[:].ap[0]), [0, H // 2], [1, 64]],
            )
            v_tiles = []
            for vi in range(2):
                vt = vpool.tile([128, LC, VW], BF16, tag=f"v{vi}", name=f"v{vi}")
                for lc in range(LC):
                    base = vt[:, lc, :]
                    for par, ooff in ((0, 64), (1, 128)):
                        dst = bass.AP(
                            tensor=base.tensor, offset=base.offset + ooff,
                            ap=[list(base.ap[0]), [256, H // 2], [1, 64]],
                        )
                        nc.sync.dma_start(dst, ones_b)
                v_tiles.append(vt)

            def _v_evac(base, ps, nh):
                """One 4D-strided cast: psum [128, 8 heads x 64] into v_sb
                head blocks (even head [v|ones] low half, odd head [ones|v]
                high half): dst dims (pair, parity, el)."""
                dst = bass.AP(
                    tensor=base.tensor,
                    offset=base.offset + nh * 8 * 128,
                    ap=[list(base.ap[0]), [256, 4], [192, 2], [1, 64]],
                )
                src = bass.AP(
                    tensor=ps.tensor,
                    offset=ps.offset,
                    ap=[list(ps.ap[0]), [128, 4], [64, 2], [1, 64]],
                )
                nc.vector.tensor_copy(dst, src)

            # ---------------- per-block tile state
            state = {}

            def make_qkvu(j):
                q_t = qkpool.tile([128, KC, L], BF16, tag="q", name=f"q{j}")
                k_t = qkpool.tile([128, KC, L], BF16, tag="k", name=f"k{j}")
                u_t = upool.tile([128, KC, L], BF16, tag="u", name=f"u{j}")
                state[j] = {"q": q_t, "k": k_t, "v": v_tiles[j % 2], "u": u_t}

            # ---------------- filler-unit generators (each yield ~= 2 MMs)
            def gen_qk(j, names=("q", "k")):
                zt_t = zt_tiles[j]
                for nm in names:
                    w_t = w_sb[nm]
                    out_t = state[j][nm]
                    for dc in range(KC):
                        ps = pspool.tile([128, 512], F32, tag="pj", bufs=2,
                                         name="pjq")
                        for kcp in range(4):
                            for kc in (2 * kcp, 2 * kcp + 1):
                                nc.tensor.matmul(
                                    ps[:],
                                    w_t[:, kc, dc * 128:(dc + 1) * 128].opt(),
                                    zt_t[:, kc, :].opt(),
                                    start=(kc == 0), stop=(kc == KC - 1),
                                )
                            yield
                        nc.vector.tensor_copy(out_t[:, dc, :], ps[:])

            def gen_v(j, lcs=tuple(range(LC))):
                zt_t = zt_tiles[j]
                v_t = state[j]["v"]
                wv_t = w_sb["v"]
                for lc in lcs:
                    base = v_t[:, lc, :]
                    for nh in range(2):
                        ps = pspool.tile([128, 512], F32, tag="pj", bufs=2,
                                         name="pjv")
                        for kcp in range(4):
                            for kc in (2 * kcp, 2 * kcp + 1):
                                nc.tensor.matmul(
                                    ps[:],
                                    zt_t[:, kc, lc * 128:(lc + 1) * 128].opt(),
                                    wv_t[:, kc, nh * 512:(nh + 1) * 512].opt(),
                                    start=(kc == 0), stop=(kc == KC - 1),
                                )
                            yield
                        _v_evac(base, ps[:], nh)
                        yield

            def gen_wo(j, lcs=tuple(range(LC)), y_on_dve=False):
                u_t = state[j]["u"]
                wo_t = w_sb["o"]
                for lc in lcs:
                    for eh in range(2):
                        ps = pspool.tile([128, 512], F32, tag="pj", bufs=2,
                                         name="pjo")
                        for dcp in range(4):
                            for dc in (2 * dcp, 2 * dcp + 1):
                                nc.tensor.matmul(
                                    ps[:],
                                    u_t[:, dc, lc * 128:(lc + 1) * 128].opt(),
                                    wo_t[:, dc, eh * 512:(eh + 1) * 512].opt(),
                                    start=(dc == 0), stop=(dc == KC - 1),
                                )
                            yield
                        y_sb = ypool.tile([128, 512], BF16, tag="y", name="ysb")
                        if y_on_dve:
                            nc.vector.tensor_copy(y_sb[:], ps[:])
                        else:
                            nc.scalar.copy(y_sb[:], ps[:])
                        nc.sync.dma_start(
                            y_d[j, lc * 128:(lc + 1) * 128,
                                eh * 512:(eh + 1) * 512],
                            y_sb[:],
                        )
                        yield

            # ---------------- attention emission with filler pumping
            def emit_attn(i, queue, prepump=0, rr=True):
                rr_state = [0]

                def pump(n):
                    for _ in range(n):
                        popped = False
                        while queue:
                            if rr:
                                rr_state[0] %= len(queue)
                            else:
                                rr_state[0] = 0
                            try:
                                next(queue[rr_state[0]])
                                rr_state[0] += 1
                                popped = True
                                break
                            except StopIteration:
                                queue.pop(rr_state[0])
                        if not popped:
                            break

                pump(prepump)
                q_t, k_t = state[i]["q"], state[i]["k"]
                v_t, u_t = state[i]["v"], state[i]["u"]
                uacc = {}

                def emit_pv(c, mc, p):
                    if mc == 0:
                        uacc[c] = (
                            pspool.tile([128, 512], F32, tag="ue", bufs=1,
                                        name="ue"),
                            pspool.tile([128, 512], F32, tag="uo", bufs=1,
                                        name="uo"),
                        )
                    ue, uo = uacc[c]
                    for par in range(2):
                        h = 2 * c + par
                        nc.tensor.matmul(
                            (ue, uo)[par][:],
                            v_t[:, mc, h * 128:(h + 1) * 128].opt(),
                            p[:, par, :].opt(),
                            start=(mc == 0), stop=(mc == LC - 1),
                        )
                    if mc != LC - 1:
                        return
                    # u evac: even head [v|ones] -> u rows 0:64, S 64:128
                    #         odd head [ones|v] -> S 0:64, u 64:128
                    tmp = rpool.tile([64, 512], F32, tag="rtmp", name="rtmp")
                    nc.vector.tensor_copy(tmp[0:64, :], ue[64:128, :])
                    r_a = rpool.tile([64, 512], F32, tag="ra", name="ra")
                    nc.vector.reciprocal_approx_fast(r_a[0:64, :], tmp[0:64, :])
                    r_b = rpool.tile([64, 512], F32, tag="rb", name="rb")
                    nc.vector.reciprocal_approx_fast(r_b[0:64, :], uo[0:64, :])
                    nc.vector.tensor_mul(
                        u_t[0:64, c, :], ue[0:64, :], r_a[0:64, :]
                    )
                    nc.vector.tensor_mul(
                        u_t[64:128, c, :], uo[64:128, :], r_b[0:64, :]
                    )
                    del uacc[c]

                pend = None
                for c in range(KC):
                    for half in range(2):
                        # burst two m-chunks back-to-back: the second step's
                        # row-tiled LDWEIGHTS hide under the first step's
                        # opposite-row-half matmul
                        burst = []
                        for mc in (2 * half, 2 * half + 1):
                            sc = pspool.tile([128, 2, 512], F32, tag="sc",
                                             bufs=2, name="sc")
                            for par in range(2):
                                h0 = par * 64
                                nc.tensor.matmul(
                                    sc[:, par, :],
                                    k_t[h0:h0 + 64, c,
                                        mc * 128:(mc + 1) * 128].opt(),
                                    q_t[h0:h0 + 64, c, :].opt(),
                                    start=True, stop=True,
                                )
                            p = ppool.tile([128, 2, 512], BF16, tag="p",
                                           name="p")
                            nc.scalar.activation(p[:], sc[:], EXP_FUNC)
                            burst.append((mc, p))
                        pump(1)
                        if pend is not None:
                            pc, pburst = pend
                            for mc, p in pburst:
                                emit_pv(pc, mc, p)
                            if pburst[-1][0] == LC - 1:
                                pump(2)  # cover the u-evac DVE chain
                        pend = (c, burst)
                        pump(2)
                pc, pburst = pend
                for mc, p in pburst:
                    emit_pv(pc, mc, p)
                # phase end: drain remaining fillers
                while queue:
                    pump(1)
                    if not queue:
                        break

            # ---------------- block pipeline
            make_qkvu(0)
            # Block-0 Q runs kc-major over all 8 PSUM banks, so the first
            # matmul only needs the first wq chunk (not the full 2MB) to
            # have landed. K/V follow dc-major (their weights arrive during
            # Q) with progressive evacuation.
            zt0 = zt_tiles[0]
            sc0 = pspool.tile([128, 2, 512], F32, tag="sc", bufs=2, name="sc")
            sc1 = pspool.tile([128, 2, 512], F32, tag="sc", bufs=2, name="sc")
            ue0 = pspool.tile([128, 512], F32, tag="ue", bufs=1, name="ue")
            uo0 = pspool.tile([128, 512], F32, tag="uo", bufs=1, name="uo")
            pj0 = pspool.tile([128, 512], F32, tag="pj", bufs=2, name="pj0")
            pj1 = pspool.tile([128, 512], F32, tag="pj", bufs=2, name="pj1")
            accs = [sc0[:, 0, :], sc0[:, 1, :], sc1[:, 0, :], sc1[:, 1, :],
                    ue0[:], uo0[:], pj0[:], pj1[:]]
            for nm in ("q", "k"):
                w_t = w_sb[nm]
                out_t = state[0][nm]
                for kc in range(KC):
                    for dc in range(KC):
                        nc.tensor.matmul(
                            accs[dc],
                            w_t[:, kc, dc * 128:(dc + 1) * 128].opt(),
                            zt0[:, kc, :].opt(),
                            start=(kc == 0), stop=(kc == KC - 1),
                        )
                        if kc == KC - 1:
                            nc.vector.tensor_copy(out_t[:, dc, :], accs[dc])
            v0 = state[0]["v"]
            wv0_t = w_sb["v"]
            for kc in range(KC):
                for lc in range(LC):
                    for nh in range(2):
                        nc.tensor.matmul(
                            accs[lc * 2 + nh],
                            zt0[:, kc, lc * 128:(lc + 1) * 128].opt(),
                            wv0_t[:, kc, nh * 512:(nh + 1) * 512].opt(),
                            start=(kc == 0), stop=(kc == KC - 1),
                        )
                        if kc == KC - 1:
                            _v_evac(v0[:, lc, :], accs[lc * 2 + nh], nh)

            for i in range(NBLK):
                if i + 1 < NBLK:
                    make_qkvu(i + 1)
                if i + 2 < NBLK:
                    load_zt(i + 2, nc.sync)
                prepump = 0
                if i == 0:
                    queue = [gen_v(1), gen_qk(1)]
                elif i == 1:
                    queue = [gen_wo(0), gen_qk(2), gen_v(2)]
                elif i == 2:
                    queue = [gen_qk(3), gen_v(3, (0, 1))]
                else:
                    queue = [gen_v(3, (2, 3)), gen_wo(1, y_on_dve=True)]
                    prepump = 12
                emit_attn(i, queue, prepump, rr=(i != 3))

            # Wo(2) streams here while attn(3)'s last u-evac completes
            for _ in gen_wo(2):
                pass
            # final Wo(3): dc-major over all 8 PSUM banks -- the first 7 dc
            # passes only need pairs 0..6, so they stream while the last
            # pair's u-evac DVE chain completes
            scA = pspool.tile([128, 2, 512], F32, tag="sc", bufs=2, name="sc")
            scB = pspool.tile([128, 2, 512], F32, tag="sc", bufs=2, name="sc")
            ueF = pspool.tile([128, 512], F32, tag="ue", bufs=1, name="ue")
            uoF = pspool.tile([128, 512], F32, tag="uo", bufs=1, name="uo")
            pjA = pspool.tile([128, 512], F32, tag="pj", bufs=2, name="pjA")
            pjB = pspool.tile([128, 512], F32, tag="pj", bufs=2, name="pjB")
            faccs = [scA[:, 0, :], scA[:, 1, :], scB[:, 0, :], scB[:, 1, :],
                     ueF[:], uoF[:], pjA[:], pjB[:]]
            u3 = state[NBLK - 1]["u"]
            wo_t = w_sb["o"]
            combos = [(lc, eh) for lc in range(LC) for eh in range(2)]
            for dc in range(KC):
                for g, (lc, eh) in enumerate(combos):
                    nc.tensor.matmul(
                        faccs[g],
                        u3[:, dc, lc * 128:(lc + 1) * 128].opt(),
                        wo_t[:, dc, eh * 512:(eh + 1) * 512].opt(),
                        start=(dc == 0), stop=(dc == KC - 1),
                    )
                    if dc == KC - 1:
                        y_sb = ypool.tile([128, 512], BF16, tag="y",
                                          name="ysb")
                        nc.scalar.copy(y_sb[:], faccs[g])
                        nc.sync.dma_start(
                            y_d[NBLK - 1, lc * 128:(lc + 1) * 128,
                                eh * 512:(eh + 1) * 512],
                            y_sb[:],
                        )

    nc.finalize()
    return nc


_NC_CACHE = {}


def _get_nc():
    if "nc" not in _NC_CACHE:
        _NC_CACHE["nc"] = _build_nc()
    return _NC_CACHE["nc"]


def _numpy_fallback(x, Wq, bq, Wk, bk, Wv, bv, Wo, bo, layer_bit):
    """Reference-equivalent numpy path (only for nonzero qkv biases,
    which the staged problem never produces)."""
    x = np.asarray(x, np.float32)
    C = N // CHUNK
    ids = np.arange(C)
    partner = ids ^ (1 << int(layer_bit))
    a_idx = ids[ids < partner]
    b_idx = partner[ids < partner]
    xr = x.reshape(B, C, CHUNK, D)
    blocks = np.concatenate([xr[:, a_idx], xr[:, b_idx]], axis=2)
    z = blocks.transpose(1, 0, 2, 3).reshape(-1, L, D)
    q = (z @ Wq + bq).reshape(-1, L, H, DH)
    k = (z @ Wk + bk).reshape(-1, L, H, DH)
    v = (z @ Wv + bv).reshape(-1, L, H, DH)
    s = np.einsum("blhd,bmhd->bhlm", q, k) / np.sqrt(DH).astype(np.float32)
    s = s - s.max(axis=-1, keepdims=True)
    p = np.exp(s)
    p /= p.sum(axis=-1, keepdims=True)
    u = np.einsum("bhlm,bmhd->blhd", p, v).reshape(-1, L, D)
    yb = u @ Wo + bo
    yb = yb.reshape(len(a_idx), B, 2, CHUNK, D)
    out = np.empty((B, C, CHUNK, D), np.float32)
    out[:, a_idx] = yb[:, :, 0].transpose(1, 0, 2, 3)
    out[:, b_idx] = yb[:, :, 1].transpose(1, 0, 2, 3)
    return out.reshape(B, N, D)


def _prep(x, Wq, Wk, Wv, Wo, layer_bit):
    x = np.asarray(x, dtype=np.float32)
    C = N // CHUNK
    ids = np.arange(C)
    partner = ids ^ (1 << int(layer_bit))
    a_idx = ids[ids < partner]
    b_idx = partner[ids < partner]
    P = a_idx.shape[0]

    xr = x.reshape(B, C, CHUNK, D)
    blocks = np.concatenate([xr[:, a_idx], xr[:, b_idx]], axis=2)  # [B,P,L,D]
    blocks = np.ascontiguousarray(
        blocks.transpose(1, 0, 3, 2).reshape(P * B, D, L).astype(ml_dtypes.bfloat16)
    )  # z^T per block, pair-major
    scale = np.float32(1.0 / np.sqrt(DH))

    bf = ml_dtypes.bfloat16
    base = {
        "wq": np.ascontiguousarray((np.asarray(Wq, np.float32) * scale).astype(bf)),
        "wk": np.ascontiguousarray(np.asarray(Wk, np.float32).astype(bf)),
        "wv": np.ascontiguousarray(np.asarray(Wv, np.float32).astype(bf)),
        "wo": np.ascontiguousarray(np.asarray(Wo, np.float32).astype(bf)),
        "ones": np.ones((128, 64), bf),
    }
    in_maps = []
    for core in range(NCORES):
        m = dict(base)
        m["zt"] = blocks[core * NBLK:(core + 1) * NBLK]
        in_maps.append(m)
    return in_maps, (a_idx, b_idx, P)


def _gather(results, idxs, bo):
    a_idx, b_idx, P = idxs
    yb = np.concatenate(
        [np.asarray(r["y"], np.float32) for r in results], axis=0
    )  # [P*B, L, D]
    yb = yb.reshape(P, B, 2, CHUNK, D)
    out = np.empty((B, N // CHUNK, CHUNK, D), np.float32)
    out[:, a_idx] = yb[:, :, 0].transpose(1, 0, 2, 3)
    out[:, b_idx] = yb[:, :, 1].transpose(1, 0, 2, 3)
    out = out.reshape(B, N, D)
    bo = np.asarray(bo, np.float32) if bo is not None else None
    if bo is not None and np.any(bo):
        out = out + bo
    return out


def _run(inputs, trace=False):
    bq, bk, bv = (inputs.get(k) for k in ("bq", "bk", "bv"))
    if any(b is not None and np.any(np.asarray(b)) for b in (bq, bk, bv)):
        zeros = np.zeros((D,), np.float32)
        out = _numpy_fallback(
            inputs["x"], inputs["Wq"],
            np.asarray(bq, np.float32) if bq is not None else zeros,
            inputs["Wk"],
            np.asarray(bk, np.float32) if bk is not None else zeros,
            inputs["Wv"],
            np.asarray(bv, np.float32) if bv is not None else zeros,
            inputs["Wo"],
            np.asarray(inputs.get("bo"), np.float32)
            if inputs.get("bo") is not None else zeros,
            inputs["layer_bit"],
        )
        return out, None
    in_maps, idxs = _prep(
        inputs["x"], inputs["Wq"], inputs["Wk"], inputs["Wv"], inputs["Wo"],
        inputs["layer_bit"],
    )
    nc = _get_nc()
    res = run_bass_kernel_spmd(nc, in_maps, list(range(NCORES)), trace=trace)
    out = _gather(res.results, idxs, inputs.get("bo"))
    return out, res


def kernel(**inputs):
    out, _ = _run(inputs, trace=False)
    return out


def kernel_traced(**inputs):
    out, res = _run(inputs, trace=True)
    return out, res
